# revision 6
# baseline (speedup 1.0000x reference)
"""Trainium2 Bass kernel for nn_CoocOpModel.

out[b,k] = sum_ij func[b,i] * C[i,j,k] * arg[b,j] + op_mask[b,k]

with B=64, V=8192, C: [V, V, 3] f32 (805 MB -> memory-bound).

Distribution: C is sharded along its first (i) axis across 8 NeuronCores
(1024 rows / ~100 MB per core); func/arg are replicated. Each core computes
the partial contraction over its local i range:

  T[b, (j,k)] = sum_i f[b,i] * C[i, (j,k)]      (TensorE, f chunks stationary,
                                                 PSUM accumulation over i-chunks)
  partial[b,k] = sum_j T[b,j,k] * a[b,j]        (DVE tensor_tensor_reduce,
                                                 strided PSUM read per k)

plus op_mask/8. The 8 per-core partials are combined with an AllGather of the
[64,3] partial (4.6 us floor vs 9.7 us for AllReduce on 8 cores) followed by a
local 8-way DVE sum, so every core ends with the full [64, 3] result.

Streaming schedule: C is consumed in col-windows of 3072 (12 KB contiguous
DMA runs) with a tapered tail (1536/1152/384) so the last window's trailing
matmul+reduce after the DMA stream ends is only ~3 us. A deep SBUF tile ring
(22 x 6 KB/partition) keeps the window-N+1 loads from ever stalling on
window-N consumption. An early dummy AllGather absorbs cross-core launch skew
so the final gather doesn't pay it.
"""

import os

import numpy as np

B = 64
V = 8192
K3 = 3
N_CORES = 8


def _build_nc(v_local, j_total, b, n_cores, tail="ag", warmup=True, bufs=22):
    import concourse.bass as bass
    import concourse.mybir as mybir
    import concourse.tile as tile
    from concourse import bacc

    f32 = mybir.dt.float32
    f16 = mybir.dt.float16
    P = 128
    IC = v_local // P            # i-chunks per core
    NK = j_total * K3            # total moving columns
    J_T = 128                    # j per psum tile
    NT = J_T * K3                # psum tile cols (384)
    assert v_local % P == 0

    # Window schedule: 3072-col windows (12 KB f32 DMA runs per partition),
    # tapered tail so post-stream trailing compute is minimal.
    wins = [3072] * 7 + [1536, 1152, 384]
    assert sum(wins) == NK and all(w % NT == 0 for w in wins)

    nc = bacc.Bacc(None, target_bir_lowering=False, debug=False,
                   num_devices=n_cores, num_swdge_queues=2)
    # f pre-blocked on host: [128, IC*b] with f_blk[p, ic*b + bb] =
    # func[bb, ic*128 + p] -> contiguous 2 KB rows (fast DMA descriptors).
    f_blk = nc.declare_dram_parameter("f_blk", [P, IC * b], f32, isOutput=False)
    arg_v = nc.declare_dram_parameter("arg_v", [b, j_total], f32,
                                      isOutput=False)
    flags = nc.declare_dram_parameter("flags", [b, 2], f32, isOutput=False)
    cooc = nc.declare_dram_parameter("cooc", [v_local, NK], f32,
                                     isOutput=False)
    out = nc.declare_dram_parameter("out", [b, K3], f32, isOutput=True)

    with tile.TileContext(nc) as tc, \
            tc.tile_pool(name="pers", bufs=1) as pers, \
            tc.tile_pool(name="cchunk", bufs=bufs) as cpool, \
            tc.tile_pool(name="psum", bufs=8, space="PSUM") as ppool, \
            tc.tile_pool(name="scr", bufs=2) as spool, \
            tc.tile_pool(name="dram", bufs=1, space="DRAM") as dpool:

        # ---- persistent inputs -------------------------------------------
        # f in fp16 (i on partitions); SWDGE casts f32->fp16 during DMA.
        # fp32 matmuls run as two PE passes on TRN2 -- fp16 inputs halve the
        # PE work while PSUM accumulation stays fp32. fp16 (10-bit mantissa)
        # keeps elementwise error ~3e-3; C~N(0,1), f in [0,1) fit the range.
        fsb = pers.tile([P, IC * b], f16)
        asb = pers.tile([b, j_total], f32)           # arg rows, b on partitions
        flg = pers.tile([b, 2], f32)
        nc.gpsimd.dma_start(out=fsb[:], in_=f_blk[:])
        half_j = j_total // 2
        nc.sync.dma_start(out=asb[:, :half_j], in_=arg_v[:, :half_j])
        nc.sync.dma_start(out=asb[:, half_j:], in_=arg_v[:, half_j:])
        nc.sync.dma_start(out=flg[:], in_=flags[:])

        slots = pers.tile([b, (NK // NT) * K3], f32)  # per-psum-tile partials
        mask8 = pers.tile([b, K3], f32)               # op_mask / n_cores
        ms = pers.tile([b, 1], f32)

        # op_mask / n_cores from the two flag columns:
        # col0 = q*a1 ; col1 = q*(a1 + a2 - a1*a2) ; col2 = 0,  q = -1e9/ncores
        q = -1.0e9 / n_cores
        nc.vector.tensor_mul(ms[:], flg[:, 0:1], flg[:, 1:2])
        nc.vector.tensor_add(mask8[:, 1:2], flg[:, 0:1], flg[:, 1:2])
        nc.vector.tensor_sub(mask8[:, 1:2], mask8[:, 1:2], ms[:])
        nc.vector.tensor_scalar_mul(mask8[:, 1:2], mask8[:, 1:2], q)
        nc.vector.tensor_scalar_mul(mask8[:, 0:1], flg[:, 0:1], q)
        nc.vector.memset(mask8[:, 2:3], 0.0)

        # ---- main streaming loop over C ----------------------------------
        # C shard viewed as [p=128, ic, n]: row i = ic*128 + p. One DMA per
        # (window, ic) moving [128, NWIN] with NWIN-col contiguous f32 runs.
        cooc_r = cooc[:].rearrange("(c p) n -> p c n", p=P)

        warm_done = False
        g = 0                                  # running global psum-tile index
        nwin0 = 0
        for cc, NWIN in enumerate(wins):
            ic_tiles = []
            for ic in range(IC):
                ct = cpool.tile([P, NWIN], f16, tag="cw",
                                name=f"cp_{cc}_{ic}")
                nc.gpsimd.dma_start(
                    out=ct[:],
                    in_=cooc_r[:, ic, nwin0:nwin0 + NWIN],
                )
                ic_tiles.append(ct)

            if warmup and not warm_done:
                # Dummy AllGather to absorb cross-core launch skew and warm
                # the collective path; runs on TOPSP+SDMA, overlapping the
                # C stream. Emitted after window 0's loads so it doesn't
                # delay the stream head.
                wb_in = dpool.tile([b, 2], f32, tag="wbin")
                wb_out = dpool.tile([b * n_cores, 2], f32, tag="wbout",
                                    addr_space="Shared")
                nc.sync.dma_start(out=wb_in[:], in_=flg[:])
                nc.gpsimd.collective_compute(
                    "AllGather",
                    mybir.AluOpType.bypass,
                    replica_groups=[list(range(n_cores))],
                    ins=[wb_in.opt()],
                    outs=[wb_out.opt()],
                )
                warm_done = True

            ntiles_w = NWIN // NT
            for t0 in range(0, ntiles_w, 4):
                tcount = min(4, ntiles_w - t0)
                ptiles = [ppool.tile([b, NT], f32, tag="pt",
                                     name=f"pt_{cc}_{t0}_{t}")
                          for t in range(tcount)]
                for ic in range(IC):
                    ct = ic_tiles[ic]
                    for t in range(tcount):
                        base = (t0 + t) * NT
                        nc.tensor.matmul(
                            out=ptiles[t][:],
                            lhsT=fsb[:, ic * b:(ic + 1) * b],
                            rhs=ct[:, base:base + NT],
                            start=(ic == 0),
                            stop=(ic == IC - 1),
                        )
                for t in range(tcount):
                    gt = g + t0 + t
                    j0 = gt * J_T
                    # prod[b,(j,k)] = T[b,(j,k)] * a[b,j] (stride-0 k bcast)
                    scr = spool.tile([b, NT], f32, tag="scr", name=f"scr_{gt}")
                    a_sl = asb[:, j0:j0 + J_T]
                    a_bc = bass.AP(a_sl.tensor, a_sl.offset,
                                   [list(a_sl.ap[0]), list(a_sl.ap[1]),
                                    [0, K3]])
                    nc.vector.tensor_mul(
                        scr[:].rearrange("p (j k) -> p j k", k=K3),
                        ptiles[t][:].rearrange("p (j k) -> p j k", k=K3),
                        a_bc,
                    )
                    # slots[b, gt, k] = sum_j prod[b, j, k]
                    nc.vector.tensor_reduce(
                        out=slots[:, gt * K3:(gt + 1) * K3],
                        in_=scr[:].rearrange("p (j k) -> p k j", k=K3),
                        axis=mybir.AxisListType.X,
                        op=mybir.AluOpType.add,
                    )
            g += ntiles_w
            nwin0 += NWIN

        # ---- fold partials + mask, gather across cores, sum, store ------
        racc = pers.tile([b, K3], f32)
        nc.vector.tensor_reduce(
            out=racc[:],
            in_=slots[:].rearrange("p (g k) -> p k g", k=K3),
            axis=mybir.AxisListType.X,
            op=mybir.AluOpType.add,
        )
        nc.vector.tensor_add(racc[:], racc[:], mask8[:])

        if tail == "none":
            # Per-core partial only; host sums the 8 partials (unshard step).
            nc.sync.dma_start(out=out[:], in_=racc[:])
        elif tail == "ag":
            bounce_in = dpool.tile([b, K3], f32, tag="bin")
            bounce_out = dpool.tile([b * n_cores, K3], f32, tag="bout",
                                    addr_space="Shared")
            nc.sync.dma_start(out=bounce_in[:], in_=racc[:])
            nc.gpsimd.collective_compute(
                "AllGather",
                mybir.AluOpType.bypass,
                replica_groups=[list(range(n_cores))],
                ins=[bounce_in.opt()],
                outs=[bounce_out.opt()],
            )
            # [r*b + p, k] -> SBUF [p, r*K3+k], then reduce over r.
            gth = pers.tile([b, n_cores * K3], f32)
            nc.sync.dma_start(
                out=gth[:].rearrange("p (r k) -> p r k", k=K3),
                in_=bounce_out[:].rearrange("(r p) k -> p r k", p=b),
            )
            res = pers.tile([b, K3], f32)
            nc.vector.tensor_reduce(
                out=res[:],
                in_=gth[:].rearrange("p (r k) -> p k r", k=K3),
                axis=mybir.AxisListType.X,
                op=mybir.AluOpType.add,
            )
            nc.sync.dma_start(out=out[:], in_=res[:])
        else:
            bounce_in = dpool.tile([b, K3], f32, tag="bin")
            bounce_out = dpool.tile([b, K3], f32, tag="bout",
                                    addr_space="Shared" if n_cores > 4
                                    else "Local")
            nc.sync.dma_start(out=bounce_in[:], in_=racc[:])
            nc.gpsimd.collective_compute(
                "AllReduce",
                mybir.AluOpType.add,
                replica_groups=[list(range(n_cores))],
                ins=[bounce_in.opt()],
                outs=[bounce_out.opt()],
            )
            nc.sync.dma_start(out=out[:], in_=bounce_out[:])

    nc.compile()
    return nc


_NC_CACHE = {}


def _get_nc(v_local, j_total, b, n_cores):
    key = (v_local, j_total, b, n_cores)
    if key not in _NC_CACHE:
        tail = os.environ.get("COOC_TAIL", "ag")
        warmup = os.environ.get("COOC_WARMUP", "1") != "0"
        bufs = int(os.environ.get("COOC_BUFS", "22"))
        _NC_CACHE[key] = _build_nc(v_local, j_total, b, n_cores,
                                   tail=tail, warmup=warmup, bufs=bufs)
    return _NC_CACHE[key]


def make_in_maps(func, arg, cooccurrences, n_cores):
    """Shard the full inputs for SPMD execution (host-side layout only)."""
    func = np.ascontiguousarray(np.asarray(func, dtype=np.float32))
    arg = np.ascontiguousarray(np.asarray(arg, dtype=np.float32))
    cooc = np.asarray(cooccurrences, dtype=np.float32)
    v = cooc.shape[0]
    v_local = v // n_cores
    P = 128
    ic = v_local // P
    f_t = np.ascontiguousarray(func[:, :v].T)            # [V, B]
    arg_v = np.ascontiguousarray(arg[:, :v])             # [B, V]
    flags = np.ascontiguousarray(func[:, v:v + 2])       # [B, 2]
    in_maps = []
    for c in range(n_cores):
        sl = slice(c * v_local, (c + 1) * v_local)
        f_blk = np.ascontiguousarray(
            f_t[sl].reshape(ic, P, -1).transpose(1, 0, 2).reshape(P, -1)
        )
        in_maps.append({
            "f_blk": f_blk,
            "arg_v": arg_v,
            "flags": flags,
            "cooc": cooc[sl].reshape(v_local, v * K3),
        })
    return in_maps


def kernel(func, arg, cooccurrences):
    from concourse.bass_utils import run_bass_kernel_spmd

    in_maps = make_in_maps(func, arg, cooccurrences, N_CORES)
    nc = _get_nc(V // N_CORES, V, B, N_CORES)
    res = run_bass_kernel_spmd(nc, in_maps, core_ids=list(range(N_CORES)))
    return np.asarray(res.results[0]["out"], dtype=np.float32)


# revision 7
# speedup vs baseline: 1.0489x; 1.0489x over previous
"""Trainium2 Bass kernel for nn_CoocOpModel.

out[b,k] = sum_ij func[b,i] * C[i,j,k] * arg[b,j] + op_mask[b,k]

with B=64, V=8192, C: [V, V, 3] f32 (805 MB -> memory-bound).

Distribution: C is sharded along its first (i) axis across 8 NeuronCores
(1024 rows / ~100 MB per core); func/arg are replicated. Each core computes
the partial contraction over its local i range:

  T[b, (j,k)] = sum_i f[b,i] * C[i, (j,k)]      (TensorE, f chunks stationary,
                                                 PSUM accumulation over i-chunks)
  partial[b,k] = sum_j T[b,j,k] * a[b,j]        (DVE tensor_tensor_reduce,
                                                 strided PSUM read per k)

plus op_mask/8. The 8 per-core partials are combined with an AllGather of the
[64,3] partial (4.6 us floor vs 9.7 us for AllReduce on 8 cores) followed by a
local 8-way DVE sum, so every core ends with the full [64, 3] result.

Streaming schedule: C is consumed in col-windows of 3072 (12 KB contiguous
DMA runs) with a tapered tail (1536/1152/384) so the last window's trailing
matmul+reduce after the DMA stream ends is only ~3 us. A deep SBUF tile ring
(22 x 6 KB/partition) keeps the window-N+1 loads from ever stalling on
window-N consumption. An early dummy AllGather absorbs cross-core launch skew
so the final gather doesn't pay it.
"""

import os

import numpy as np

B = 64
V = 8192
K3 = 3
N_CORES = 8


def _build_nc(v_local, j_total, b, n_cores, tail="ag", warmup=True, bufs=22):
    import concourse.bass as bass
    import concourse.mybir as mybir
    import concourse.tile as tile
    from concourse import bacc

    f32 = mybir.dt.float32
    f16 = mybir.dt.float16
    P = 128
    IC = v_local // P            # i-chunks per core
    NK = j_total * K3            # total moving columns
    J_T = 128                    # j per psum tile
    NT = J_T * K3                # psum tile cols (384)
    assert v_local % P == 0

    # Window schedule: 3072-col windows (12 KB f32 DMA runs per partition),
    # tapered tail so post-stream trailing compute is minimal.
    wins = [3072] * 7 + [1536, 1152, 384]
    assert sum(wins) == NK and all(w % NT == 0 for w in wins)

    nc = bacc.Bacc(None, target_bir_lowering=False, debug=False,
                   num_devices=n_cores, num_swdge_queues=2)
    # f pre-blocked on host: [128, IC*b] with f_blk[p, ic*b + bb] =
    # func[bb, ic*128 + p] -> contiguous 2 KB rows (fast DMA descriptors).
    f_blk = nc.declare_dram_parameter("f_blk", [P, IC * b], f32, isOutput=False)
    arg_v = nc.declare_dram_parameter("arg_v", [b, j_total], f32,
                                      isOutput=False)
    flags = nc.declare_dram_parameter("flags", [b, 2], f32, isOutput=False)
    cooc = nc.declare_dram_parameter("cooc", [v_local, NK], f32,
                                     isOutput=False)
    out = nc.declare_dram_parameter("out", [b, K3], f32, isOutput=True)

    with tile.TileContext(nc) as tc, \
            tc.tile_pool(name="pers", bufs=1) as pers, \
            tc.tile_pool(name="cchunk", bufs=bufs) as cpool, \
            tc.tile_pool(name="psum", bufs=8, space="PSUM") as ppool, \
            tc.tile_pool(name="scr", bufs=2) as spool, \
            tc.tile_pool(name="dram", bufs=1, space="DRAM") as dpool:

        # ---- persistent inputs -------------------------------------------
        # f in fp16 (i on partitions); SWDGE casts f32->fp16 during DMA.
        # fp32 matmuls run as two PE passes on TRN2 -- fp16 inputs halve the
        # PE work while PSUM accumulation stays fp32. fp16 (10-bit mantissa)
        # keeps elementwise error ~3e-3; C~N(0,1), f in [0,1) fit the range.
        fsb = pers.tile([P, IC * b], f16)
        asb = pers.tile([b, j_total], f32)           # arg rows, b on partitions
        flg = pers.tile([b, 2], f32)
        nc.gpsimd.dma_start(out=fsb[:], in_=f_blk[:])
        half_j = j_total // 2
        nc.sync.dma_start(out=asb[:, :half_j], in_=arg_v[:, :half_j])
        nc.sync.dma_start(out=asb[:, half_j:], in_=arg_v[:, half_j:])
        nc.sync.dma_start(out=flg[:], in_=flags[:])

        slots = pers.tile([b, (NK // NT) * K3], f32)  # per-psum-tile partials
        mask8 = pers.tile([b, K3], f32)               # op_mask / n_cores
        ms = pers.tile([b, 1], f32)

        # op_mask / n_cores from the two flag columns:
        # col0 = q*a1 ; col1 = q*(a1 + a2 - a1*a2) ; col2 = 0,  q = -1e9/ncores
        q = -1.0e9 / n_cores
        nc.vector.tensor_mul(ms[:], flg[:, 0:1], flg[:, 1:2])
        nc.vector.tensor_add(mask8[:, 1:2], flg[:, 0:1], flg[:, 1:2])
        nc.vector.tensor_sub(mask8[:, 1:2], mask8[:, 1:2], ms[:])
        nc.vector.tensor_scalar_mul(mask8[:, 1:2], mask8[:, 1:2], q)
        nc.vector.tensor_scalar_mul(mask8[:, 0:1], flg[:, 0:1], q)
        nc.vector.memset(mask8[:, 2:3], 0.0)

        # ---- main streaming loop over C ----------------------------------
        # C shard viewed as [p=128, ic, n]: row i = ic*128 + p. One DMA per
        # (window, ic) moving [128, NWIN] with NWIN-col contiguous f32 runs.
        cooc_r = cooc[:].rearrange("(c p) n -> p c n", p=P)

        warm_done = False
        g = 0                                  # running global psum-tile index
        nwin0 = 0
        for cc, NWIN in enumerate(wins):
            ic_tiles = []
            for ic in range(IC):
                ct = cpool.tile([P, NWIN], f16, tag="cw",
                                name=f"cp_{cc}_{ic}")
                nc.gpsimd.dma_start(
                    out=ct[:],
                    in_=cooc_r[:, ic, nwin0:nwin0 + NWIN],
                )
                ic_tiles.append(ct)

            if warmup and not warm_done:
                # Dummy AllGather to absorb cross-core launch skew and warm
                # the collective path; runs on TOPSP+SDMA, overlapping the
                # C stream. Emitted after window 0's loads so it doesn't
                # delay the stream head.
                wb_in = dpool.tile([b, 2], f32, tag="wbin")
                wb_out = dpool.tile([b * n_cores, 2], f32, tag="wbout",
                                    addr_space="Shared")
                nc.sync.dma_start(out=wb_in[:], in_=flg[:])
                nc.gpsimd.collective_compute(
                    "AllGather",
                    mybir.AluOpType.bypass,
                    replica_groups=[list(range(n_cores))],
                    ins=[wb_in.opt()],
                    outs=[wb_out.opt()],
                )
                warm_done = True

            ntiles_w = NWIN // NT
            for t0 in range(0, ntiles_w, 4):
                tcount = min(4, ntiles_w - t0)
                ptiles = [ppool.tile([b, NT], f32, tag="pt",
                                     name=f"pt_{cc}_{t0}_{t}")
                          for t in range(tcount)]
                for ic in range(IC):
                    ct = ic_tiles[ic]
                    for t in range(tcount):
                        base = (t0 + t) * NT
                        nc.tensor.matmul(
                            out=ptiles[t][:],
                            lhsT=fsb[:, ic * b:(ic + 1) * b],
                            rhs=ct[:, base:base + NT],
                            start=(ic == 0),
                            stop=(ic == IC - 1),
                        )
                for t in range(tcount):
                    gt = g + t0 + t
                    j0 = gt * J_T
                    # prod[b,(j,k)] = T[b,(j,k)] * a[b,j] (stride-0 k bcast)
                    scr = spool.tile([b, NT], f32, tag="scr", name=f"scr_{gt}")
                    a_sl = asb[:, j0:j0 + J_T]
                    a_bc = bass.AP(a_sl.tensor, a_sl.offset,
                                   [list(a_sl.ap[0]), list(a_sl.ap[1]),
                                    [0, K3]])
                    nc.vector.tensor_mul(
                        scr[:].rearrange("p (j k) -> p j k", k=K3),
                        ptiles[t][:].rearrange("p (j k) -> p j k", k=K3),
                        a_bc,
                    )
                    # slots[b, gt, k] = sum_j prod[b, j, k]
                    nc.vector.tensor_reduce(
                        out=slots[:, gt * K3:(gt + 1) * K3],
                        in_=scr[:].rearrange("p (j k) -> p k j", k=K3),
                        axis=mybir.AxisListType.X,
                        op=mybir.AluOpType.add,
                    )
            g += ntiles_w
            nwin0 += NWIN

        # ---- fold partials + mask, gather across cores, sum, store ------
        racc = pers.tile([b, K3], f32)
        nc.vector.tensor_reduce(
            out=racc[:],
            in_=slots[:].rearrange("p (g k) -> p k g", k=K3),
            axis=mybir.AxisListType.X,
            op=mybir.AluOpType.add,
        )
        nc.vector.tensor_add(racc[:], racc[:], mask8[:])

        if tail == "none":
            # Per-core partial only; host sums the 8 partials (unshard step).
            nc.sync.dma_start(out=out[:], in_=racc[:])
        elif tail == "ag":
            bounce_in = dpool.tile([b, K3], f32, tag="bin")
            bounce_out = dpool.tile([b * n_cores, K3], f32, tag="bout",
                                    addr_space="Shared")
            nc.sync.dma_start(out=bounce_in[:], in_=racc[:])
            nc.gpsimd.collective_compute(
                "AllGather",
                mybir.AluOpType.bypass,
                replica_groups=[list(range(n_cores))],
                ins=[bounce_in.opt()],
                outs=[bounce_out.opt()],
            )
            # [r*b + p, k] -> SBUF [p, r*K3+k], then reduce over r.
            gth = pers.tile([b, n_cores * K3], f32)
            nc.sync.dma_start(
                out=gth[:].rearrange("p (r k) -> p r k", k=K3),
                in_=bounce_out[:].rearrange("(r p) k -> p r k", p=b),
            )
            res = pers.tile([b, K3], f32)
            nc.vector.tensor_reduce(
                out=res[:],
                in_=gth[:].rearrange("p (r k) -> p k r", k=K3),
                axis=mybir.AxisListType.X,
                op=mybir.AluOpType.add,
            )
            nc.sync.dma_start(out=out[:], in_=res[:])
        else:
            bounce_in = dpool.tile([b, K3], f32, tag="bin")
            bounce_out = dpool.tile([b, K3], f32, tag="bout",
                                    addr_space="Shared" if n_cores > 4
                                    else "Local")
            nc.sync.dma_start(out=bounce_in[:], in_=racc[:])
            nc.gpsimd.collective_compute(
                "AllReduce",
                mybir.AluOpType.add,
                replica_groups=[list(range(n_cores))],
                ins=[bounce_in.opt()],
                outs=[bounce_out.opt()],
            )
            nc.sync.dma_start(out=out[:], in_=bounce_out[:])

    nc.compile()
    return nc


_NC_CACHE = {}


def _get_nc(v_local, j_total, b, n_cores):
    key = (v_local, j_total, b, n_cores)
    if key not in _NC_CACHE:
        tail = os.environ.get("COOC_TAIL", "ag")
        warmup = os.environ.get("COOC_WARMUP", "1") != "0"
        bufs = int(os.environ.get("COOC_BUFS", "22"))
        _NC_CACHE[key] = _build_nc(v_local, j_total, b, n_cores,
                                   tail=tail, warmup=warmup, bufs=bufs)
    return _NC_CACHE[key]


def make_in_maps(func, arg, cooccurrences, n_cores):
    """Shard the full inputs for SPMD execution (host-side layout only)."""
    func = np.ascontiguousarray(np.asarray(func, dtype=np.float32))
    arg = np.ascontiguousarray(np.asarray(arg, dtype=np.float32))
    cooc = np.asarray(cooccurrences, dtype=np.float32)
    v = cooc.shape[0]
    v_local = v // n_cores
    P = 128
    ic = v_local // P
    f_t = np.ascontiguousarray(func[:, :v].T)            # [V, B]
    arg_v = np.ascontiguousarray(arg[:, :v])             # [B, V]
    flags = np.ascontiguousarray(func[:, v:v + 2])       # [B, 2]
    in_maps = []
    for c in range(n_cores):
        sl = slice(c * v_local, (c + 1) * v_local)
        f_blk = np.ascontiguousarray(
            f_t[sl].reshape(ic, P, -1).transpose(1, 0, 2).reshape(P, -1)
        )
        in_maps.append({
            "f_blk": f_blk,
            "arg_v": arg_v,
            "flags": flags,
            "cooc": cooc[sl].reshape(v_local, v * K3),
        })
    return in_maps


def combine_results(res):
    """Unshard: every core holds the full [B,3] result unless tail='none',
    in which case the 8 per-core partials are summed here (host unshard)."""
    if os.environ.get("COOC_TAIL", "ag") == "none":
        parts = [np.asarray(r["out"], dtype=np.float32) for r in res.results]
        return np.sum(np.stack(parts, 0), axis=0, dtype=np.float32)
    return np.asarray(res.results[0]["out"], dtype=np.float32)


def kernel(func, arg, cooccurrences):
    from concourse.bass_utils import run_bass_kernel_spmd

    in_maps = make_in_maps(func, arg, cooccurrences, N_CORES)
    nc = _get_nc(V // N_CORES, V, B, N_CORES)
    res = run_bass_kernel_spmd(nc, in_maps, core_ids=list(range(N_CORES)))
    return combine_results(res)
